# revision 1
# baseline (speedup 1.0000x reference)
# Trainium2 Bass kernel for nn_BatchelorAdj (motion-compensated MRI recon adjoint).
#
# Math:  out = sum_t W_t^T( sum_c conj(S_c) . IFFT2c(K_c . M_ct) )
#   - IFFT2c(X) == A @ X @ A with A = P F^-1 P (P = fftshift perm, A symmetric),
#     run as float32r matmuls (full fp32 precision, full PE rate at N=320).
#   - W_t^T (adjoint bilinear warp) == sum over 16x8 source tiles of banded
#     matmuls (Ex*im)^T @ Ey where Ex[q,j] = relu(1-|j - pxr_q|) is the exact
#     bilinear weight (triangular hat), built with one subtract + Abs + Relu.
#
# Sharding: 8 cores; core r does frames [3r,3r+1,3r+2] fully, plus coils
# [2r,2r+1] of frame 24 (warp is linear in the image, so per-core partial coil
# sums warp independently and everything adds in the final host-side reduce).
import math
import numpy as np

Nx = Ny = 320
Nc = 16
Nt = 25
NCORES = 8
BX, BY = 32, 4              # warp source tile (BX*BY = 128 = one K chunk)
NTX, NTY = Nx // BX, Ny // BY
NTILE = NTX * NTY           # 800
FR_FULL = 3                 # full frames per core
C24 = Nc // NCORES          # coils of frame 24 per core
NSLOT = FR_FULL + 1

_CACHE = {}


def _build_A():
    j = np.arange(Nx)
    F = np.exp(2j * np.pi * np.outer(j, j) / Nx) / np.sqrt(Nx)
    P = np.zeros((Nx, Nx))
    P[j, (j + Nx // 2) % Nx] = 1.0
    A = P @ F @ P
    return A.real.astype(np.float32), A.imag.astype(np.float32)


def _chunk3(arr2d):
    """[320, W] -> [3, 128, W] zero-padded."""
    out = np.zeros((3, 128, arr2d.shape[1]), dtype=arr2d.dtype)
    out[0] = arr2d[0:128]
    out[1] = arr2d[128:256]
    out[2, :64] = arr2d[256:320]
    return out


def _build_program(D, debug_dump=False):
    from concourse import bass, bacc, tile, mybir

    JX = BX + 2 * D + 1
    JY = BY + 2 * D + 1
    PWW = BY * (NTY - 1) + JY          # psum band width (357 for D=18)
    f32 = mybir.dt.float32
    f32r = mybir.dt.float32r
    bf16 = mybir.dt.bfloat16
    MULT = mybir.AluOpType.mult
    ADD = mybir.AluOpType.add
    SUB = mybir.AluOpType.subtract
    ACTF = mybir.ActivationFunctionType

    nc = bacc.Bacc("TRN2", target_bir_lowering=False, debug=False,
                   num_devices=NCORES)

    # ---- DRAM tensors (SPMD: same shapes on all cores, per-core values) ----
    ksT = nc.dram_tensor("ksT", [Nc, 2, 3, 128, Nx], f32, kind="ExternalInput")
    ks24T = nc.dram_tensor("ks24T", [C24, 2, 3, 128, Nx], f32, kind="ExternalInput")
    maskT = nc.dram_tensor("maskT", [FR_FULL, Nc, 3, 128, Nx], bf16, kind="ExternalInput")
    mask24T = nc.dram_tensor("mask24T", [C24, 3, 128, Nx], bf16, kind="ExternalInput")
    smg = nc.dram_tensor("smg", [Nc, 2, 3, 128, Ny], f32, kind="ExternalInput")
    sm24g = nc.dram_tensor("sm24g", [C24, 2, 3, 128, Ny], f32, kind="ExternalInput")
    Acst = nc.dram_tensor("Acst", [3, 3, 128, Ny], f32r, kind="ExternalInput")
    pxrd = nc.dram_tensor("pxrd", [NSLOT, 128, NTILE], f32, kind="ExternalInput")
    pyrd = nc.dram_tensor("pyrd", [NSLOT, 128, NTILE], f32, kind="ExternalInput")
    iotaxd = nc.dram_tensor("iotaxd", [128, JX], f32, kind="ExternalInput")
    iotayd = nc.dram_tensor("iotayd", [128, JY], f32, kind="ExternalInput")
    zzd = nc.dram_tensor("zzd", [1, 512], f32, kind="ExternalInput")
    outp = nc.dram_tensor("outp", [2, 3, 128, Ny], f32, kind="ExternalOutput")
    if debug_dump:
        dbg_aux = nc.dram_tensor("dbg_aux", [2, 3, 128, Ny], f32, kind="ExternalOutput")
        dbg_imc = nc.dram_tensor("dbg_imc", [2, 128, NTILE], f32, kind="ExternalOutput")
        dbg_t1 = nc.dram_tensor("dbg_t1", [2, 3, 128, Ny], f32, kind="ExternalOutput")

    from contextlib import ExitStack
    with tile.TileContext(nc) as tc, ExitStack() as ctx:
        const_pool = ctx.enter_context(tc.tile_pool(name="const", bufs=1))
        acc_pool = ctx.enter_context(tc.tile_pool(name="acc", bufs=1))
        aux_pool = ctx.enter_context(tc.tile_pool(name="aux", bufs=2))
        ks_pool = ctx.enter_context(tc.tile_pool(name="ks", bufs=2))
        mk_pool = ctx.enter_context(tc.tile_pool(name="mk", bufs=3))
        km_pool = ctx.enter_context(tc.tile_pool(name="km", bufs=2))
        t1_pool = ctx.enter_context(tc.tile_pool(name="t1", bufs=2))
        sm_pool = ctx.enter_context(tc.tile_pool(name="sm", bufs=2))
        cmb_pool = ctx.enter_context(tc.tile_pool(name="cmb", bufs=4))
        imc_pool = ctx.enter_context(tc.tile_pool(name="imc", bufs=2))
        pxy_pool = ctx.enter_context(tc.tile_pool(name="pxy", bufs=1))
        ex_pool = ctx.enter_context(tc.tile_pool(name="ex", bufs=2))
        ey_pool = ctx.enter_context(tc.tile_pool(name="ey", bufs=2))
        exim_pool = ctx.enter_context(tc.tile_pool(name="exim", bufs=2))
        fl_pool = ctx.enter_context(tc.tile_pool(name="fl", bufs=2))
        psum_fft = ctx.enter_context(tc.tile_pool(name="psf", bufs=6, space="PSUM"))
        psum_warp = ctx.enter_context(tc.tile_pool(name="psw", bufs=1, space="PSUM"))

        # ---- constants to SBUF ----
        A_sb = []
        for w in range(3):
            t = const_pool.tile([128, 3, Ny], f32r, tag=f"A{w}")
            nc.sync.dma_start(t[:, :, :], Acst.ap()[w].transpose([1, 0, 2]))
            A_sb.append(t)
        iotax_sb = const_pool.tile([128, JX], f32, tag="iox")
        nc.sync.dma_start(iotax_sb[:, :], iotaxd.ap()[:, :])
        iotay_sb = const_pool.tile([128, JY], f32, tag="ioy")
        nc.sync.dma_start(iotay_sb[:, :], iotayd.ap()[:, :])
        zz_sb = const_pool.tile([1, 512], f32, tag="zz")
        nc.sync.dma_start(zz_sb[:, :], zzd.ap()[:, :])
        zzh_sb = const_pool.tile([1, 512], bf16, tag="zzh")
        nc.vector.tensor_copy(zzh_sb[:, :], zz_sb[:, :])

        out_acc = []
        for comp in range(2):
            t = acc_pool.tile([128, 3, Ny], f32, tag=f"oacc{comp}")
            nc.vector.memset(t[:, :, :], 0.0)
            out_acc.append(t)

        AR, AI, NAI = 0, 1, 2

        # ---- FFT + coil-combine; coils OUTER so kspace/smaps stream twice
        # per core instead of once per frame; slots processed in pairs to
        # bound live aux accumulators ----

        def fft_combine(ksbt, smbt, mk_src, auxp):
            msb = mk_pool.tile([128, 3, Nx], bf16, tag="msb")
            nc.sync.dma_start(msb[:, :, :], mk_src.transpose([1, 0, 2]))
            kmTt = []
            for comp in range(2):
                km = km_pool.tile([128, 3, Nx], f32r, tag=f"km{comp}",
                                  name=f"km_{comp}")
                nc.gpsimd.tensor_tensor(km[:, :, :], ksbt[comp][:, :, :],
                                        msb[:, :, :], MULT)
                kmTt.append(km)

            # pass 1: T1 = km @ A  (T1[x,u], stored [p, m, u])
            T1 = [t1_pool.tile([128, 3, Ny], f32r, tag=f"T1{comp}",
                               name=f"T1_{comp}")
                  for comp in range(2)]
            for m in range(3):
                mc = 128 if m < 2 else 64
                for comp, terms in ((0, ((0, AR), (1, NAI))),
                                    (1, ((0, AI), (1, AR)))):
                    ps = psum_fft.tile([128, Ny], f32, tag="psf", name="psf")
                    i = 0
                    for (kcomp, w) in terms:
                        for ky in range(3):
                            kp = 128 if ky < 2 else 64
                            nc.tensor.matmul(
                                ps[0:mc, :],
                                kmTt[kcomp][0:kp, ky, 128 * m:128 * m + mc],
                                A_sb[w][0:kp, ky, :],
                                start=(i == 0), stop=(i == 5))
                            i += 1
                    nc.vector.tensor_copy(T1[comp][0:mc, m, :], ps[0:mc, :])

            # pass 2: im = A @ T1 ; combine with conj(smaps)
            for m in range(3):
                mc = 128 if m < 2 else 64
                psv = []
                for comp, terms in ((0, ((0, AR), (1, NAI))),
                                    (1, ((1, AR), (0, AI)))):
                    ps = psum_fft.tile([128, Ny], f32, tag="psf", name="psf2")
                    i = 0
                    for (tcomp, w) in terms:
                        for kx in range(3):
                            kp = 128 if kx < 2 else 64
                            nc.tensor.matmul(
                                ps[0:mc, :],
                                A_sb[w][0:kp, kx, 128 * m:128 * m + mc],
                                T1[tcomp][0:kp, kx, :],
                                start=(i == 0), stop=(i == 5))
                            i += 1
                    psv.append(ps)
                # aux_r += sr*imr + si*imi ; aux_i += sr*imi - si*imr
                for (ocomp, scomp, icomp, op) in ((0, 0, 0, ADD), (0, 1, 1, ADD),
                                                 (1, 0, 1, ADD), (1, 1, 0, SUB)):
                    p = cmb_pool.tile([128, Ny], f32, tag="cmb", name="cmb")
                    nc.vector.tensor_tensor(p[0:mc, :], smbt[scomp][0:mc, m, :],
                                            psv[icomp][0:mc, :], MULT)
                    nc.vector.tensor_tensor(auxp[ocomp][0:mc, m, :],
                                            auxp[ocomp][0:mc, m, :], p[0:mc, :], op)

        def load_ks_sm(ks_src, sm_src):
            ksbt, smbt = [], []
            for comp in range(2):
                ksb = ks_pool.tile([128, 3, Nx], f32, tag=f"ksb{comp}",
                                   name=f"ksb_{comp}")
                nc.sync.dma_start(ksb[:, :, :], ks_src[comp].transpose([1, 0, 2]))
                ksbt.append(ksb)
                smb = sm_pool.tile([128, 3, Ny], f32, tag=f"smb{comp}",
                                   name=f"smb_{comp}")
                nc.sync.dma_start(smb[:, :, :], sm_src[comp].transpose([1, 0, 2]))
                smbt.append(smb)
            return ksbt, smbt

        for grp in ([0, 1], [2, 3]):
          aux = {}
          for slot in grp:
            pair = []
            for comp in range(2):
                t = aux_pool.tile([128, 3, Ny], f32, tag=f"aux{slot % 2}{comp}",
                                  name=f"aux_{slot}_{comp}")
                nc.vector.memset(t[:, :, :], 0.0)
                pair.append(t)
            aux[slot] = pair
          for c in range(Nc):
            ksbt, smbt = load_ks_sm(ksT.ap()[c], smg.ap()[c])
            for slot in grp:
                if slot < FR_FULL:
                    fft_combine(ksbt, smbt, maskT.ap()[slot, c], aux[slot])
          if 3 in grp:
            for c in range(C24):
                ksbt, smbt = load_ks_sm(ks24T.ap()[c], sm24g.ap()[c])
                fft_combine(ksbt, smbt, mask24T.ap()[c], aux[3])

          # ---- phase 2: adjoint warp per slot ----
          for slot in grp:
            if debug_dump and slot == 0:
                for comp in range(2):
                    nc.sync.dma_start(dbg_aux.ap()[comp].transpose([1, 0, 2]),
                                      aux[slot][comp][:, :, :])
            imc = []
            for comp in range(2):
                t = imc_pool.tile([128, NTILE], f32, tag=f"imc{comp}",
                                  name=f"imc_{comp}")
                with nc.allow_non_contiguous_dma(reason="strided imc gather"):
                    for a in range(NTX):
                        k, p0 = (32 * a) // 128, (32 * a) % 128
                        rs = aux[slot][comp][p0:p0 + 32, k, :].rearrange(
                            "p (g ul) -> p g ul", g=NTY, ul=BY)
                        for ul in range(BY):
                            nc.sync.dma_start(
                                t[32 * ul:32 * ul + 32, NTY * a:NTY * a + NTY],
                                rs[:, :, ul])
                imc.append(t)
            if debug_dump and slot == 0:
                for comp in range(2):
                    nc.sync.dma_start(dbg_imc.ap()[comp], imc[comp][:, :])

            pxr_sb = pxy_pool.tile([128, NTILE], f32, tag="pxr")
            nc.sync.dma_start(pxr_sb[:, :], pxrd.ap()[slot])
            pyr_sb = pxy_pool.tile([128, NTILE], f32, tag="pyr")
            nc.sync.dma_start(pyr_sb[:, :], pyrd.ap()[slot])

            for bx in range(NTX):
                pw = []
                for comp in range(2):
                    t = psum_warp.tile([JX, PWW], f32, tag=f"pw{comp}",
                                       name=f"pw_{comp}")
                    nc.tensor.matmul(t[:, :], zzh_sb[0:1, 0:JX], zzh_sb[0:1, 0:PWW],
                                     start=True, stop=False, skip_group_check=True)
                    pw.append(t)

                NH = NTY // 2     # construction sub-chunk (SBUF pressure)
                for h in range(2):
                    c0 = NTY * bx + NH * h
                    eng = nc.gpsimd if h == 0 else nc.vector
                    ex = ex_pool.tile([128, NH, JX], f32, tag="ex")
                    eng.tensor_tensor(
                        ex[:, :, :],
                        iotax_sb[:, :].unsqueeze(1).broadcast_to([128, NH, JX]),
                        pxr_sb[:, c0:c0 + NH].unsqueeze(2).broadcast_to([128, NH, JX]),
                        SUB)
                    ey = ey_pool.tile([128, NH, JY], f32, tag="ey")
                    eng.tensor_tensor(
                        ey[:, :, :],
                        iotay_sb[:, :].unsqueeze(1).broadcast_to([128, NH, JY]),
                        pyr_sb[:, c0:c0 + NH].unsqueeze(2).broadcast_to([128, NH, JY]),
                        SUB)
                    nc.scalar.activation(ex[:, :, :], ex[:, :, :], ACTF.Abs)
                    nc.scalar.activation(ey[:, :, :], ey[:, :, :], ACTF.Abs)
                    nc.scalar.activation(ex[:, :, :], ex[:, :, :], ACTF.Relu,
                                         scale=-1.0, bias=1.0)
                    nc.scalar.activation(ey[:, :, :], ey[:, :, :], ACTF.Relu,
                                         scale=-1.0, bias=1.0)
                    eyim = []
                    for comp in range(2):
                        t = exim_pool.tile([128, NH, JY], f32, tag=f"eyim{comp}",
                                           name=f"eyim_{comp}")
                        e2 = nc.vector if comp == 0 else nc.gpsimd
                        e2.tensor_tensor(
                            t[:, :, :], ey[:, :, :],
                            imc[comp][:, c0:c0 + NH].unsqueeze(2)
                            .broadcast_to([128, NH, JY]),
                            MULT)
                        eyim.append(t)

                    for ti in range(NH):
                        y0 = BY * (NH * h + ti)
                        for comp in range(2):
                            nc.tensor.matmul(
                                pw[comp][:, y0:y0 + JY],
                                ex[:, ti, :],
                                eyim[comp][:, ti, :],
                                start=False,
                                stop=(h == 1 and ti == NH - 1),
                                skip_group_check=True)

                # ---- flush band: rows [BX*bx - D, BX*bx + BX + D] ----
                g0 = BX * bx - D
                r0, r1 = max(0, g0), min(Nx, g0 + JX)
                for comp in range(2):
                    tmp = fl_pool.tile([JX, Ny], f32, tag="fl")
                    nc.vector.tensor_copy(tmp[0:JX, :], pw[comp][0:JX, D:D + Ny])
                    ra = r0
                    while ra < r1:
                        k = ra // 128
                        rb = min(r1, 128 * (k + 1))
                        pa, pb = ra - 128 * k, rb - 128 * k
                        tmp2 = fl_pool.tile([128, Ny], f32, tag="fl2")
                        nc.gpsimd.memset(tmp2[:, :], 0.0)
                        nc.sync.dma_start(tmp2[pa:pb, :], tmp[ra - g0:rb - g0, :])
                        nc.vector.tensor_tensor(
                            out_acc[comp][:, k, :],
                            out_acc[comp][:, k, :], tmp2[:, :], ADD)
                        ra = rb

        for comp in range(2):
            nc.sync.dma_start(outp.ap()[comp].transpose([1, 0, 2]),
                              out_acc[comp][:, :, :])

    nc.compile()
    return nc


def _host_prep(kspace_r, kspace_i, mask, smaps_r, smaps_i, flow, D):
    f32 = np.float32
    import ml_dtypes
    bf16 = ml_dtypes.bfloat16
    JX = BX + 2 * D + 1
    JY = BY + 2 * D + 1

    Ar, Ai = _build_A()
    Acst = np.stack([_chunk3(Ar), _chunk3(Ai), _chunk3(-Ai)])  # [3,3,128,320]

    # kspace transposed [c, comp, ychunk, p, x]
    kT = np.stack([kspace_r.transpose(2, 1, 0), kspace_i.transpose(2, 1, 0)], 1)
    ksT = np.zeros((Nc, 2, 3, 128, Nx), f32)
    ksT[:, :, 0] = kT[:, :, 0:128]
    ksT[:, :, 1] = kT[:, :, 128:256]
    ksT[:, :, 2, :64] = kT[:, :, 256:320]

    # mask transposed [t, c, ychunk, p, x] bf16
    mT = mask.transpose(3, 2, 1, 0)  # [t, c, y, x]
    maskT = np.zeros((Nt, Nc, 3, 128, Nx), bf16)
    maskT[:, :, 0] = mT[:, :, 0:128].astype(bf16)
    maskT[:, :, 1] = mT[:, :, 128:256].astype(bf16)
    maskT[:, :, 2, :64] = mT[:, :, 256:320].astype(bf16)

    # smaps natural [c, comp, vchunk, p, u]
    sT = np.stack([smaps_r.transpose(2, 0, 1), smaps_i.transpose(2, 0, 1)], 1)
    smg = np.zeros((Nc, 2, 3, 128, Ny), f32)
    smg[:, :, 0] = sT[:, :, 0:128]
    smg[:, :, 1] = sT[:, :, 128:256]
    smg[:, :, 2, :64] = sT[:, :, 256:320]

    # warp fields: pxr[t, q, tile] = px - BX*bx + D in compact tile layout
    X, Y = np.meshgrid(np.arange(Nx, dtype=f32), np.arange(Ny, dtype=f32),
                       indexing="ij")
    pxr_all = np.zeros((Nt, 128, NTILE), f32)
    pyr_all = np.zeros((Nt, 128, NTILE), f32)
    bxg = np.repeat(np.arange(NTX), NTY).reshape(1, NTILE)  # tile -> bx
    byg = np.tile(np.arange(NTY), NTX).reshape(1, NTILE)
    for t in range(Nt):
        px = np.clip(X + flow[:, :, 0, t], 0.0, Nx - 1.0)
        py = np.clip(Y + flow[:, :, 1, t], 0.0, Ny - 1.0)
        # [bx, xin, by, yin] -> [q = xin*BY + yin, tile = bx*NTY + by]
        # q = vl + 32*ul  (vl = v%BX, ul = u%BY) -> dims order (ul, vl)
        pxc = px.reshape(NTX, BX, NTY, BY).transpose(3, 1, 0, 2).reshape(128, NTILE)
        pyc = py.reshape(NTX, BX, NTY, BY).transpose(3, 1, 0, 2).reshape(128, NTILE)
        pxr_all[t] = pxc - BX * bxg + D
        pyr_all[t] = pyc - BY * byg + D
    assert pxr_all.min() >= 0 and pxr_all.max() <= JX - 1 + 1e-3
    assert pyr_all.min() >= 0 and pyr_all.max() <= JY - 1 + 1e-3

    iotax = np.tile(np.arange(JX, dtype=f32), (128, 1))
    iotay = np.tile(np.arange(JY, dtype=f32), (128, 1))
    zz = np.zeros((1, 512), f32)

    in_maps = []
    for r in range(NCORES):
        fr = [FR_FULL * r + s for s in range(FR_FULL)]
        cs = [C24 * r + j for j in range(C24)]
        in_maps.append({
            "ksT": ksT,
            "ks24T": np.ascontiguousarray(ksT[cs]),
            "maskT": np.ascontiguousarray(maskT[fr]),
            "mask24T": np.ascontiguousarray(maskT[Nt - 1, cs]),
            "smg": smg,
            "sm24g": np.ascontiguousarray(smg[cs]),
            "Acst": Acst,
            "pxrd": np.ascontiguousarray(pxr_all[fr + [Nt - 1]]),
            "pyrd": np.ascontiguousarray(pyr_all[fr + [Nt - 1]]),
            "iotaxd": iotax,
            "iotayd": iotay,
            "zzd": zz,
        })
    return in_maps


def kernel(kspace_r, kspace_i, mask, smaps_r, smaps_i, flow):
    from concourse.bass_utils import run_bass_kernel_spmd

    D = max(17, int(math.ceil(np.abs(flow).max())))
    if D not in _CACHE:
        _CACHE[D] = _build_program(D)
    nc = _CACHE[D]

    in_maps = _host_prep(np.asarray(kspace_r, np.float32),
                         np.asarray(kspace_i, np.float32),
                         np.asarray(mask, np.float32),
                         np.asarray(smaps_r, np.float32),
                         np.asarray(smaps_i, np.float32),
                         np.asarray(flow, np.float32), D)

    res = run_bass_kernel_spmd(nc, in_maps, core_ids=list(range(NCORES)))

    acc = np.zeros((2, Nx, Ny), np.float64)
    for r in range(NCORES):
        o = res.results[r]["outp"].astype(np.float64)  # [2, 3, 128, 320]
        for comp in range(2):
            acc[comp, 0:128] += o[comp, 0]
            acc[comp, 128:256] += o[comp, 1]
            acc[comp, 256:320] += o[comp, 2, :64]
    return np.stack([acc[0], acc[1]], axis=-1).astype(np.float32)



# revision 6
# speedup vs baseline: 2.4291x; 2.4291x over previous
# Trainium2 Bass kernel for nn_BatchelorAdj (motion-compensated MRI recon
# adjoint):  out = sum_t W_t^T( sum_c conj(S_c) . IFFT2c(K_c . M_ct) )
#
# v2 design (cost-model driven):
#  - host precomputes km = kspace*mask per (coil,frame) in a stacked-chunk
#    bf16 layout so each IFFT matmul pass runs 5-instruction chains over a
#    640-long stacked contraction ([kmr;kmi] x [A_r;-A_i]); pass 1 merges
#    the 64-row x-tail of the r/i outputs into one 128-row chain.
#  - coil combine: Act copies psums to SBUF bf16; DVE does 2 wide mults +
#    4 accumulate adds at 2-byte (2x) rate into bf16 aux accumulators.
#  - adjoint warp: host ships bf16 bilinear hat tables (ex: (32+2D)-wide
#    x-band per bx; ey: (2D+5)-wide y-window at column offset 4*by), so the
#    warp is PE work at 1 cycle/row plus one eyim multiply per (bx,comp).
#  - per-bx psum bands [128, 324+2D] are flushed with one aligned DVE add
#    into per-bx SBUF accumulators; host applies x-shifts in the final
#    cross-core reduce.
# Sharding: core r owns frames [3r,3r+1,3r+2] plus coils [2r,2r+1] of
# frame 24 (the warp is linear in the image, so partial coil sums warp
# independently).
import math
import numpy as np

Nx = Ny = 320
Nc = 16
Nt = 25
NCORES = 8
BX, BY = 32, 4
NTX, NTY = Nx // BX, Ny // BY       # 10, 80
FR_FULL = 3
C24 = Nc // NCORES                  # 2
NSLOT = FR_FULL + 1                 # 4
NJOB = FR_FULL * Nc + C24           # 50 fft jobs per core
KMW = 2176                          # km tile width (see _km_pack)

_CACHE = {}


def _build_A():
    j = np.arange(Nx)
    F = np.exp(2j * np.pi * np.outer(j, j) / Nx) / np.sqrt(Nx)
    P = np.zeros((Nx, Nx))
    P[j, (j + Nx // 2) % Nx] = 1.0
    A = P @ F @ P
    return A.real.astype(np.float64), A.imag.astype(np.float64)


def _stack5(M, N):
    """Chunk the 640-row stack [M;N] of two 320-row mats into [5,128,cols]:
    (M0, N0, M1, N1, [M2;N2])."""
    out = np.zeros((5, 128, M.shape[1]), np.float64)
    out[0] = M[0:128]
    out[1] = N[0:128]
    out[2] = M[128:256]
    out[3] = N[128:256]
    out[4, 0:64] = M[256:320]
    out[4, 64:128] = N[256:320]
    return out


def _km_pack(kmr, kmi):
    """km [y,x] pair -> [128, 1920] layout.
    cols [0:1600]:   km5 chunks (kmr0,kmi0,kmr1,kmi1,[kmr2;kmi2]) full x.
    cols [1600:1664]: [kmi2 ; -kmr2] @ x 256:320 (partition-stacked)
    cols [1664:1920]: (kmi-y0, -kmr-y0, kmi-y1, -kmr-y1) @ x 256:320
    cols [1920:2176]: (kmr-y0, kmi-y0, kmr-y1, kmi-y1) @ x 256:320."""
    out = np.zeros((128, KMW), np.float64)
    km5 = _stack5(kmr, kmi)
    out[:, 0:1600] = km5.transpose(1, 0, 2).reshape(128, 1600)
    out[0:64, 1600:1664] = kmi[256:320, 256:320]
    out[64:128, 1600:1664] = -kmr[256:320, 256:320]
    out[:, 1664:1728] = kmi[0:128, 256:320]
    out[:, 1728:1792] = -kmr[0:128, 256:320]
    out[:, 1792:1856] = kmi[128:256, 256:320]
    out[:, 1856:1920] = -kmr[128:256, 256:320]
    out[:, 1920:1984] = kmr[0:128, 256:320]
    out[:, 1984:2048] = kmi[0:128, 256:320]
    out[:, 2048:2112] = kmr[128:256, 256:320]
    out[:, 2112:2176] = kmi[128:256, 256:320]
    return out


def _build_program(D):
    from concourse import bass, bacc, tile, mybir

    W = 32 + 2 * D                     # ex table width == band rows used
    JYU = 2 * D + 5                    # ey table width (uniform window)
    PWW = Ny + 2 * D + 4               # psum band columns (y + D offset)
    f32 = mybir.dt.float32
    bf16 = mybir.dt.bfloat16
    MULT = mybir.AluOpType.mult
    ADD = mybir.AluOpType.add
    SUB = mybir.AluOpType.subtract
    ACTF = mybir.ActivationFunctionType

    nc = bacc.Bacc("TRN2", target_bir_lowering=False, debug=False,
                   num_devices=NCORES)

    kmX = nc.dram_tensor("kmX", [NJOB, 128, KMW], bf16, kind="ExternalInput")
    smX = nc.dram_tensor("smX", [Nc + C24, 128, 3, 3, Ny], bf16,
                         kind="ExternalInput")
    ASd = nc.dram_tensor("ASd", [2, 5, 128, Ny], bf16, kind="ExternalInput")
    ASPd = nc.dram_tensor("ASPd", [2, 5, 128, 128], bf16,
                          kind="ExternalInput")
    exd = nc.dram_tensor("exd", [NSLOT, NTX, 128, NTY, W], bf16,
                         kind="ExternalInput")
    eyd = nc.dram_tensor("eyd", [NSLOT, NTX, 128, JYU, NTY], bf16,
                         kind="ExternalInput")
    zzd = nc.dram_tensor("zzd", [1, 512], bf16, kind="ExternalInput")
    outp = nc.dram_tensor("outp", [NTX, 2, 128, Ny], f32,
                          kind="ExternalOutput")

    from contextlib import ExitStack
    with tile.TileContext(nc) as tc, ExitStack() as ctx:
        const_pool = ctx.enter_context(tc.tile_pool(name="const", bufs=1))
        acc_pool = ctx.enter_context(tc.tile_pool(name="acc", bufs=1))
        aux_pool = ctx.enter_context(tc.tile_pool(name="aux", bufs=1))
        sm_pool = ctx.enter_context(tc.tile_pool(name="sm", bufs=2))
        km_pool = ctx.enter_context(tc.tile_pool(name="km", bufs=2))
        t1_pool = ctx.enter_context(tc.tile_pool(name="t1", bufs=2))
        im_pool = ctx.enter_context(tc.tile_pool(name="im", bufs=2))
        p_pool = ctx.enter_context(tc.tile_pool(name="prod", bufs=2))
        mid_pool = ctx.enter_context(tc.tile_pool(name="mid", bufs=2))
        imc_pool = ctx.enter_context(tc.tile_pool(name="imc", bufs=1))
        ex_pool = ctx.enter_context(tc.tile_pool(name="ex", bufs=2))
        ey_pool = ctx.enter_context(tc.tile_pool(name="ey", bufs=2))
        eyim_pool = ctx.enter_context(tc.tile_pool(name="eyim", bufs=2))
        psum_p1 = ctx.enter_context(tc.tile_pool(name="ps1", bufs=2,
                                                 space="PSUM"))
        psum_p2 = ctx.enter_context(tc.tile_pool(name="ps2", bufs=2,
                                                 space="PSUM"))
        psum_pw = ctx.enter_context(tc.tile_pool(name="psw", bufs=2,
                                                 space="PSUM"))

        # ---- constants ----
        AS = []
        for s in range(2):
            t = const_pool.tile([128, 5, Ny], bf16, tag=f"AS{s}")
            nc.sync.dma_start(t[:, :, :], ASd.ap()[s].transpose([1, 0, 2]))
            AS.append(t)
        ASP = []
        for s in range(2):
            t = const_pool.tile([128, 5, 128], bf16, tag=f"ASP{s}")
            nc.sync.dma_start(t[:, :, :], ASPd.ap()[s].transpose([1, 0, 2]))
            ASP.append(t)
        zz = const_pool.tile([1, 512], bf16, tag="zz")
        nc.sync.dma_start(zz[:, :], zzd.ap()[:, :])

        acc = {}
        for bx in range(NTX):
            for comp in range(2):
                t = acc_pool.tile([128, Ny], f32, tag=f"acc{bx}_{comp}")
                nc.gpsimd.memset(t[:, :], 0.0)
                acc[(bx, comp)] = t

        # ---- one FFT + coil-combine job ----
        def fft_job(job_id, smt, auxt):
            km = km_pool.tile([128, KMW], bf16, tag="km", name=f"km{job_id}")
            nc.sync.dma_start(km[:, :], kmX.ap()[job_id])
            km5 = km[:, 0:1600].rearrange("p (c x) -> p c x", c=5, x=Ny)

            # pass 1: 4 plain chains + merged x-tail chain -> T1S [128,5,320]
            t1s = t1_pool.tile([128, 5, Ny], bf16, tag="t1s",
                               name=f"t1s{job_id}")
            for (c5, mlo, aset) in ((0, 0, 0), (1, 0, 1),
                                    (2, 128, 0), (3, 128, 1)):
                ps = psum_p1.tile([128, Ny], f32, tag="p1", name="p1")
                for c in range(5):
                    nc.tensor.matmul(ps[:, :], km5[:, c, mlo:mlo + 128],
                                     AS[aset][:, c, :],
                                     start=(c == 0), stop=(c == 4))
                nc.scalar.activation(t1s[:, c5, :], ps[:, :], ACTF.Copy)
            ps = psum_p1.tile([128, Ny], f32, tag="p1", name="p1m2")
            m2lhs = [
                km[:, 1920:2048],
                km[:, 1664:1792],
                km[:, 2048:2176],
                km[:, 1792:1920],
                km[:, 1536:1664],
            ]
            for c in range(5):
                nc.tensor.matmul(ps[:, :], m2lhs[c], AS[0][:, c, :],
                                 start=(c == 0), stop=(c == 4))
            nc.scalar.activation(t1s[:, 4, :], ps[:, :], ACTF.Copy)

            # pass 2: 6 chains -> im [128, 2, 3, 320] bf16
            imt = im_pool.tile([128, 2, 3, Ny], bf16, tag="im",
                               name=f"im{job_id}")
            for comp in range(2):
                for m in range(3):
                    ps = psum_p2.tile([128, Ny], f32, tag="p2", name="p2")
                    for c in range(5):
                        if m < 2:
                            lhs = AS[comp][:, c, 128 * m:128 * m + 128]
                        else:
                            lhs = ASP[comp][:, c, :]
                        nc.tensor.matmul(ps[:, :], lhs, t1s[:, c, :],
                                         start=(c == 0), stop=(c == 4))
                    nc.scalar.activation(imt[:, comp, m, :], ps[:, :],
                                         ACTF.Copy)

            # coil combine: P = (sr*imr, si*imi, si*imr, sr*imi)
            P = p_pool.tile([128, 4, 3, Ny], bf16, tag="P", name=f"P{job_id}")
            nc.vector.tensor_tensor(P[:, 0:2, :, :], smt[:, 0:2, :, :],
                                    imt[:, 0:2, :, :], MULT)
            nc.vector.tensor_tensor(P[:, 2:4, :, :], smt[:, 1:3, :, :],
                                    imt[:, 0:2, :, :], MULT)
            nc.vector.tensor_tensor(auxt[:, 0, :, :], auxt[:, 0, :, :],
                                    P[:, 0, :, :], ADD)
            nc.vector.tensor_tensor(auxt[:, 0, :, :], auxt[:, 0, :, :],
                                    P[:, 1, :, :], ADD)
            nc.vector.tensor_tensor(auxt[:, 1, :, :], auxt[:, 1, :, :],
                                    P[:, 3, :, :], ADD)
            nc.vector.tensor_tensor(auxt[:, 1, :, :], auxt[:, 1, :, :],
                                    P[:, 2, :, :], SUB)

        def load_sm(idx):
            t = sm_pool.tile([128, 3, 3, Ny], bf16, tag="sm", name=f"sm{idx}")
            nc.sync.dma_start(t[:, :, :, :], smX.ap()[idx])
            return t

        # ---- warp chunks for one slot ----
        def warp_chunks(slot, auxt, imcs):
            def prep():
                for comp in range(2):
                    mid = mid_pool.tile([128, 3, 4, NTY], bf16, tag="mid",
                                        name=f"mid{slot}_{comp}")
                    for k in range(3):
                        nc.scalar.activation(
                            mid[:, k, :, :],
                            auxt[:, comp, k, :].rearrange(
                                "p (g ul) -> p g ul", g=NTY, ul=4)
                            .transpose([0, 2, 1]),
                            ACTF.Copy)
                    imc = imcs[comp]
                    with nc.allow_non_contiguous_dma(reason="imc gather"):
                        for a in range(NTX):
                            k, a4 = a // 4, a % 4
                            nc.sync.dma_start(
                                imc[:, NTY * a:NTY * a + NTY],
                                mid[32 * a4:32 * a4 + 32, k, :, :]
                                .rearrange("p ul g -> p (ul g)"))

            def mk_bx(bx):
                def chunk():
                    ext = ex_pool.tile([128, NTY, W], bf16, tag="ex",
                                       name=f"ex{slot}_{bx}")
                    nc.sync.dma_start(ext[:, :, :], exd.ap()[slot, bx])
                    eyt = ey_pool.tile([128, JYU, NTY], bf16, tag="ey",
                                       name=f"ey{slot}_{bx}")
                    nc.sync.dma_start(eyt[:, :, :], eyd.ap()[slot, bx])
                    pw = []
                    eyim = []
                    for comp in range(2):
                        t = eyim_pool.tile([128, JYU, NTY], bf16,
                                           tag=f"eyim{comp}",
                                           name=f"eyim{slot}_{bx}_{comp}")
                        nc.vector.tensor_tensor(
                            t[:, :, :], eyt[:, :, :],
                            imcs[comp][:, NTY * bx:NTY * bx + NTY]
                            .unsqueeze(1).broadcast_to([128, JYU, NTY]),
                            MULT)
                        eyim.append(t)
                        p = psum_pw.tile([128, PWW], f32, tag=f"pw{comp}",
                                         name=f"pw{slot}_{bx}_{comp}")
                        nc.tensor.matmul(p[:, :], zz[0:1, 0:128],
                                         zz[0:1, 0:PWW], start=True,
                                         stop=False, skip_group_check=True)
                        pw.append(p)
                    for ti in range(NTY):
                        for comp in range(2):
                            nc.tensor.matmul(
                                pw[comp][0:W, 4 * ti:4 * ti + JYU],
                                ext[:, ti, :], eyim[comp][:, :, ti],
                                start=False, stop=(ti == NTY - 1),
                                skip_group_check=True)
                    for comp in range(2):
                        nc.vector.tensor_tensor(
                            acc[(bx, comp)][:, :], acc[(bx, comp)][:, :],
                            pw[comp][:, D:D + Ny], ADD)
                return chunk
            return [prep] + [mk_bx(bx) for bx in range(NTX)]

        # ---- schedule ----
        aux = {}
        imcs_all = {}
        for slot in range(NSLOT):
            t = aux_pool.tile([128, 2, 3, Ny], bf16, tag=f"aux{slot}",
                              name=f"aux{slot}")
            nc.gpsimd.memset(t[:, :, :, :], 0.0)
            aux[slot] = t
            imcs_all[slot] = [
                imc_pool.tile([128, NTX * NTY], bf16, tag=f"imc{slot}_{c}",
                              name=f"imc{slot}_{c}") for c in range(2)]

        job = 0
        for c in range(Nc):                       # grp0: slots 0,1
            smt = load_sm(c)
            for slot in (0, 1):
                fft_job(job, smt, aux[slot])
                job += 1
        # grp1 (slot 2 + frame-24 jobs) interleaved with warps of slots 0,1
        pend = warp_chunks(0, aux[0], imcs_all[0]) + \
            warp_chunks(1, aux[1], imcs_all[1])
        pi = 0
        for c in range(Nc):
            smt = load_sm(c)
            fft_job(job, smt, aux[2])
            job += 1
            if c < C24:
                smt24 = load_sm(Nc + c)
                fft_job(job, smt24, aux[3])
                job += 1
            goal = ((c + 1) * len(pend) + Nc - 1) // Nc
            while pi < min(goal, len(pend)):
                pend[pi]()
                pi += 1
        while pi < len(pend):
            pend[pi]()
            pi += 1
        assert job == NJOB
        for slot in (2, 3):                       # tail warps
            for ch in warp_chunks(slot, aux[slot], imcs_all[slot]):
                ch()

        for bx in range(NTX):
            for comp in range(2):
                nc.sync.dma_start(outp.ap()[bx, comp],
                                  acc[(bx, comp)][:, :])

    nc.compile()
    return nc


def _host_prep(kspace_r, kspace_i, mask, smaps_r, smaps_i, flow, D):
    import ml_dtypes
    bf16 = ml_dtypes.bfloat16
    f64 = np.float64
    W = 32 + 2 * D
    JYU = 2 * D + 5

    Ar, Ai = _build_A()
    ASr = _stack5(Ar, -Ai)
    ASi = _stack5(Ai, Ar)
    ASd = np.stack([ASr, ASi]).astype(bf16)
    ASPd = np.zeros((2, 5, 128, 128), f64)
    ASPd[:, :, :, 0:64] = np.stack([ASr, ASi])[:, :, :, 256:320]
    ASPd = ASPd.astype(bf16)

    ksr = kspace_r.astype(f64)
    ksi = kspace_i.astype(f64)
    maskf = mask.astype(f64)

    def jobs_for_core(r):
        fr = [FR_FULL * r + s for s in range(FR_FULL)]
        out = []
        for c in range(Nc):
            out.append((c, fr[0]))
            out.append((c, fr[1]))
        for c in range(Nc):
            out.append((c, fr[2]))
            if c < C24:
                out.append((C24 * r + c, Nt - 1))
        return out

    # smaps [128, 3 planes (sr, si, sr), 3 m, 320]; m2 rows 64:128 zeroed
    smT = np.zeros((Nc, 128, 3, 3, Ny), f64)
    for c in range(Nc):
        for m in range(3):
            rows = min(128, Nx - 128 * m)
            smT[c, 0:rows, 0, m, :] = smaps_r[128 * m:128 * m + rows, :, c]
            smT[c, 0:rows, 1, m, :] = smaps_i[128 * m:128 * m + rows, :, c]
            smT[c, 0:rows, 2, m, :] = smaps_r[128 * m:128 * m + rows, :, c]

    # hat tables per frame
    X, Y = np.meshgrid(np.arange(Nx, dtype=f64), np.arange(Ny, dtype=f64),
                       indexing="ij")
    ex_all = {}
    ey_all = {}
    used_frames = set(range(Nt))
    for t in used_frames:
        px = np.clip(X + flow[:, :, 0, t].astype(f64), 0.0, Nx - 1.0)
        py = np.clip(Y + flow[:, :, 1, t].astype(f64), 0.0, Ny - 1.0)
        pxt = px.reshape(NTX, BX, NTY, BY)   # [bx, vl, by, ul]
        pyt = py.reshape(NTX, BX, NTY, BY)
        ex_t = np.zeros((NTX, 128, NTY, W), np.float32)
        ey_t = np.zeros((NTX, 128, JYU, NTY), np.float32)
        for bx in range(NTX):
            pxq = pxt[bx].transpose(0, 2, 1).reshape(128, NTY)  # q=4vl+ul
            pyq = pyt[bx].transpose(0, 2, 1).reshape(128, NTY)
            xrow = (32 * bx - D) + np.arange(W, dtype=f64)
            ex_t[bx] = np.maximum(
                0.0, 1.0 - np.abs(xrow[None, None, :] - pxq[:, :, None])
            ).astype(np.float32)
            by = np.arange(NTY)
            yrow = (4 * by - D)[None, :, None] + \
                np.arange(JYU, dtype=f64)[None, None, :]
            eyv = np.maximum(0.0, 1.0 - np.abs(yrow - pyq[:, :, None]))
            ey_t[bx] = eyv.astype(np.float32).transpose(0, 2, 1)
        ex_all[t] = ex_t.astype(bf16)
        ey_all[t] = ey_t.astype(bf16)

    zz = np.zeros((1, 512), bf16)

    in_maps = []
    for r in range(NCORES):
        jl = jobs_for_core(r)
        kmX = np.zeros((NJOB, 128, KMW), bf16)
        for ji, (c, t) in enumerate(jl):
            kmr = (ksr[:, :, c] * maskf[:, :, c, t]).T   # [y, x]
            kmi = (ksi[:, :, c] * maskf[:, :, c, t]).T
            kmX[ji] = _km_pack(kmr, kmi).astype(bf16)
        smXc = np.zeros((Nc + C24, 128, 3, 3, Ny), f64)
        smXc[0:Nc] = smT
        for c in range(C24):
            smXc[Nc + c] = smT[C24 * r + c]
        smXc[:, 64:128, :, 2, :] = 0.0
        slot_frames = [3 * r, 3 * r + 1, 3 * r + 2, Nt - 1]
        in_maps.append({
            "kmX": kmX,
            "smX": smXc.astype(bf16),
            "ASd": ASd,
            "ASPd": ASPd,
            "exd": np.stack([ex_all[t] for t in slot_frames]),
            "eyd": np.stack([ey_all[t] for t in slot_frames]),
            "zzd": zz,
        })
    return in_maps


def kernel(kspace_r, kspace_i, mask, smaps_r, smaps_i, flow):
    from concourse.bass_utils import run_bass_kernel_spmd

    D = int(math.ceil(np.abs(flow).max()))
    if D not in _CACHE:
        _CACHE[D] = _build_program(D)
    nc = _CACHE[D]

    in_maps = _host_prep(
        np.asarray(kspace_r, np.float32), np.asarray(kspace_i, np.float32),
        np.asarray(mask, np.float32), np.asarray(smaps_r, np.float32),
        np.asarray(smaps_i, np.float32), np.asarray(flow, np.float32), D)

    res = run_bass_kernel_spmd(nc, in_maps, core_ids=list(range(NCORES)))

    out = np.zeros((2, Nx, Ny), np.float64)
    for r in range(NCORES):
        o = res.results[r]["outp"].astype(np.float64)   # [NTX, 2, 128, 320]
        for bx in range(NTX):
            x0 = 32 * bx - D
            qlo = max(0, -x0)
            qhi = min(128, Nx - x0, 32 + 2 * D)
            for comp in range(2):
                out[comp, x0 + qlo:x0 + qhi, :] += o[bx, comp, qlo:qhi, :]
    return np.stack([out[0], out[1]], axis=-1).astype(np.float32)
